# revision 42
# baseline (speedup 1.0000x reference)
"""Trainium2 Bass kernel for multi-head attention (B=2, L=2048, D=1024, H=16).

Sharding: 8 cores = 2 (batch) x 4 (head-groups of 4 heads).  Each core
computes q/k/v projections for its 4 heads, softmax attention, and a
partial output projection against its 256 columns of W_o.  The all-reduce
of the 4 partials per batch happens on the host (free).

v12: the exp stream on ACT (~133us busy) paces the kernel.  Each of 128
score units (pair m, key-tile kt, query-chunk qc) packs BOTH heads of the
pair into one [128,1024] PSUM tile -- even head on PE row-group 0 writing
cols 0-511, odd head on row-group 64 writing cols 512-1023 (K=64 matmuls
run concurrently on the two array halves; tile_position auto-derived from
base partitions) -- then one 1024-wide exp.  ~0.8us of PV / projection /
output chain work fills the PE between units.  x arrives as 4 contiguous
column-chunk tensors so the first score fires at ~6us.  PV keeps the
ones-column trick (out row 64 = softmax denominators).
"""

import sys

if "/opt/trn_rl_repo" not in sys.path:
    sys.path.insert(0, "/opt/trn_rl_repo")

import numpy as np
import ml_dtypes

import concourse.bass as bass
import concourse.mybir as mybir
import concourse.tile as tile
from concourse import bacc
from concourse.bass_utils import run_bass_kernel_spmd

B, L, D, H = 2, 2048, 1024, 16
HD = D // H          # 64 head dim
NH = 4               # heads per core
GW = NH * HD         # 256 group width
SCALE = (H / D) ** 0.5  # 1/8
P = 128
KT = D // P          # 8 contraction tiles over D
TBLK = L // P        # 16 token blocks of 128
QC = L // 512        # 4 query chunks of 512
BF16 = mybir.dt.bfloat16
F32 = mybir.dt.float32
EXP = mybir.ActivationFunctionType.Exp

PEXP_BUFS = 40       # [P,1024] pp tiles (1 per unit; consumed <=20 units later)


def _build():
    nc = bacc.Bacc(None, target_bir_lowering=False, debug=False)

    # Host-packed inputs: one row-contiguous DRAM tensor per SBUF tile so
    # each loads with a single dma_start of 4KB+ per-partition descriptors.
    xq_d = [nc.dram_tensor(f"xq{c}", (P, KT * 512), BF16, kind="ExternalInput")
            for c in range(QC)]
    wq_d = nc.dram_tensor("wqp", (P, KT * GW), BF16, kind="ExternalInput")
    wk_d = nc.dram_tensor("wkp", (P, KT * GW), BF16, kind="ExternalInput")
    wv_d = nc.dram_tensor("wvp", (P, KT * GW), BF16, kind="ExternalInput")
    wo_d = nc.dram_tensor("wop", (P, 2 * D), BF16, kind="ExternalInput")
    out_d = nc.dram_tensor("out", (L, D), BF16, kind="ExternalOutput")

    with tile.TileContext(nc) as tc:
        with (
            tc.tile_pool(name="persist", bufs=1) as pers,
            tc.tile_pool(name="pexp", bufs=PEXP_BUFS) as pexp,
            tc.tile_pool(name="oeT", bufs=4) as oep,
            tc.tile_pool(name="rcp", bufs=2) as rcpp,
            tc.tile_pool(name="srow", bufs=2) as srp,
            tc.tile_pool(name="osb", bufs=3) as osbp,
            tc.tile_pool(name="spsum", bufs=2, space="PSUM") as sps,
            tc.tile_pool(name="accp", bufs=2, space="PSUM") as accp,
        ):
            # ---- persistent SBUF tensors ----
            # xQ[c][:, k*512:(k+1)*512] = x^T rows k*128..+128, cols c*512..+512
            xQ = [pers.tile([P, KT * 512], BF16, tag=f"xQ{c}", name=f"xQ{c}")
                  for c in range(QC)]
            wqA = pers.tile([P, KT * GW], BF16, tag="wqA")
            wkA = pers.tile([P, KT * GW], BF16, tag="wkA")
            wvA = pers.tile([P, KT * GW], BF16, tag="wvA")
            woA = pers.tile([P, 2 * D], BF16, tag="woA")
            qT = [pers.tile([P, L], BF16, tag=f"qT{m}", name=f"qT{m}") for m in range(GW // P)]
            kTt = [pers.tile([P, L], BF16, tag=f"kT{m}", name=f"kT{m}") for m in range(GW // P)]
            vext = [pers.tile([P, NH * (HD + 1)], BF16, tag=f"vx{t}", name=f"vx{t}") for t in range(TBLK)]
            aoT = [pers.tile([P, L], BF16, tag=f"aoT{m}", name=f"aoT{m}") for m in range(GW // P)]
            ones64 = pers.tile([1, HD], BF16, tag="ones64")
            nc.any.memset(ones64[:], 1.0)
            warm = pers.tile([1, 2], BF16, tag="warm")
            nc.scalar.activation(warm[:], ones64[:, 0:2], EXP)  # preload exp table
            wrow = pers.tile([1, 512], BF16, tag="wrow")
            nc.vector.memset(wrow[:], 1.0)
            # ones columns of vext (col 64 per head): constant, disjoint from
            # the v-data the chains write -- set them all during the DMA wait
            for t in range(TBLK):
                vv = vext[t][:].rearrange("p (h e) -> p h e", h=NH)
                nc.vector.memset(vv[:, :, HD:HD + 1], 1.0)

            # ---- input DMA: 9 wide transfers; xq0 split in half so the
            # prefix q/k chains start on k-tiles 0-3 at half-arrival ----
            nc.sync.dma_start(wqA[:], wq_d[:])
            nc.sync.dma_start(wkA[:], wk_d[:])
            nc.sync.dma_start(xQ[0][:, 0:4 * 512], xq_d[0][:, 0:4 * 512])
            nc.sync.dma_start(xQ[0][:, 4 * 512:], xq_d[0][:, 4 * 512:])
            for c in range(1, QC):
                nc.sync.dma_start(xQ[c][:], xq_d[c][:])
            nc.sync.dma_start(wvA[:], wv_d[:])
            nc.sync.dma_start(woA[:], wo_d[:])

            # pp tiles: pp2[(m, kt, qc)] -> [128, 1024] (even head cols 0-511)
            pp2 = {}
            pv_ps = {}   # open PV chains: (h, qc) -> psum tile, +"next" kt
            pj_ps = {}   # open projection chains

            # ---- emitters ----
            def emit_S(m, kt, qc):
                """Both heads' scores for (kt, qc) into one PSUM tile, 1 exp."""
                ps = sps.tile([P, 1024], F32, tag="sc", name=f"sc{m}_{kt}_{qc}")
                for e in range(2):
                    off = e * HD
                    nc.tensor.matmul(
                        ps[:, e * 512:(e + 1) * 512],
                        lhsT=kTt[m][off:off + HD, kt * P:(kt + 1) * P],
                        rhs=qT[m][off:off + HD, qc * 512:(qc + 1) * 512],
                        start=True,
                        stop=True,
                    )
                pp = pexp.tile([P, 1024], BF16, tag="pp", name=f"pp{m}_{kt}_{qc}")
                nc.scalar.activation(pp[:], ps[:], EXP, scale=SCALE)
                pp2[(m, kt, qc)] = pp

            def emit_proj_piece(kind, dst, w, m, tck, k0, k1):
                """Piece [k0,k1) of an 8-matmul projection chain; evict at end."""
                key = (kind, m, tck)
                ps = pj_ps.get(key)
                if ps is None:
                    ps = accp.tile([P, 512], F32, tag="wk", name=f"pj_{kind}{m}_{tck}")
                    pj_ps[key] = ps
                for k in range(k0, k1):
                    nc.tensor.matmul(
                        ps[:],
                        lhsT=w[:, k * GW + m * P:k * GW + (m + 1) * P],
                        rhs=xQ[tck][:, k * 512:(k + 1) * 512],
                        start=(k == 0),
                        stop=(k == KT - 1),
                    )
                if k1 == KT:
                    nc.vector.tensor_copy(dst[m][:, tck * 512:(tck + 1) * 512], ps[:])
                    del pj_ps[key]

            def emit_v_chain(t):
                ps = accp.tile([P, 512], F32, tag="wk", name=f"vc{t}")
                c, tc = t // 4, t % 4
                for k in range(KT):
                    nc.tensor.matmul(
                        ps[:, :GW],
                        lhsT=xQ[c][:, k * 512 + tc * P:k * 512 + (tc + 1) * P],
                        rhs=wvA[:, k * GW:(k + 1) * GW],
                        start=(k == 0),
                        stop=(k == KT - 1),
                    )
                vv = vext[t][:].rearrange("p (h e) -> p h e", h=NH)
                pv = ps[:, :GW].rearrange("p (h e) -> p h e", h=NH)
                nc.vector.tensor_copy(vv[:, :, 0:HD], pv)

            def emit_pv_steps(targets, kt_hi):
                """Advance PV chains (h, qc) to key-tile kt_hi (exclusive)."""
                for h, qc in targets:
                    start_kt = pv_ps.get((h, qc, "next"), 0)
                    if start_kt >= kt_hi:
                        continue
                    ov = pv_ps.get((h, qc))
                    if ov is None:
                        ov = accp.tile([HD + 1, 512], F32, tag="pv",
                                       name=f"ov{h}_{qc}")
                        pv_ps[(h, qc)] = ov
                    for k in range(start_kt, kt_hi):
                        nc.tensor.matmul(
                            ov[:],
                            lhsT=vext[k][:, h * (HD + 1):(h + 1) * (HD + 1)],
                            rhs=pp2[(h // 2, k, qc)][:, (h % 2) * 512:(h % 2 + 1) * 512],
                            start=(k == 0),
                            stop=(k == TBLK - 1),
                        )
                    pv_ps[(h, qc, "next")] = kt_hi

            norm_st = {}

            def emit_norm_pre(h, qc, act=False):
                """DVE half of the normalization: evict ov, build 1/sums row.
                Split from the br matmul so the PE queue never head-blocks on
                the DVE reciprocal chain."""
                ov = pv_ps.pop((h, qc))
                pv_ps.pop((h, qc, "next"), None)
                oe = oep.tile([HD, 512], BF16, tag="oe")
                if act:
                    nc.scalar.copy(oe[:], ov[0:HD, :])
                else:
                    nc.vector.tensor_copy(oe[:], ov[0:HD, :])
                srow = srp.tile([1, 512], F32, tag="s")
                nc.vector.tensor_copy(srow[:], ov[HD:HD + 1, :])
                rr = rcpp.tile([1, 512], F32, tag="r")
                nc.vector.reciprocal_approx_fast(rr[:], srow[:])
                rrb = rcpp.tile([1, 512], BF16, tag="rb")
                nc.vector.tensor_copy(rrb[:], rr[:])
                norm_st[(h, qc)] = (oe, rrb)

            def emit_norm_post(h, qc):
                """PE broadcast of 1/sums + the final DVE multiply into aoT."""
                oe, rrb = norm_st.pop((h, qc))
                m, off = h // 2, (h % 2) * HD
                br = accp.tile([HD, 512], F32, tag="wk", name=f"br{h}_{qc}")
                nc.tensor.matmul(br[:], lhsT=ones64[:], rhs=rrb[:], start=True, stop=True)
                nc.vector.tensor_mul(
                    aoT[m][off:off + HD, qc * 512:(qc + 1) * 512],
                    oe[:],
                    br[:],
                )

            def emit_norm(h, qc, act=False):
                emit_norm_pre(h, qc, act=act)
                emit_norm_post(h, qc)

            def emit_oproj(t, evict_act=False, split_dma=False, tag="wk"):
                ob = osbp.tile([P, D], BF16, tag="ob")
                for oc in range(2):
                    ps = accp.tile([P, 512], F32, tag=tag, name=f"op{t}_{oc}")
                    for i in range(GW // P):
                        nc.tensor.matmul(
                            ps[:],
                            lhsT=aoT[i][:, t * P:(t + 1) * P],
                            rhs=woA[:, i * D + oc * 512:i * D + (oc + 1) * 512],
                            start=(i == 0),
                            stop=(i == GW // P - 1),
                        )
                    if evict_act == "both":
                        if oc == 0:
                            nc.scalar.copy(ob[:, 0:512], ps[:])
                        else:
                            nc.vector.tensor_copy(ob[:, 512:1024], ps[:])
                    elif evict_act:
                        nc.scalar.copy(ob[:, oc * 512:(oc + 1) * 512], ps[:])
                    else:
                        nc.vector.tensor_copy(ob[:, oc * 512:(oc + 1) * 512], ps[:])
                    if split_dma:
                        for g in range(2):
                            nc.sync.dma_start(
                                out_d[t * P + g * 64:t * P + (g + 1) * 64,
                                      oc * 512:(oc + 1) * 512],
                                ob[g * 64:(g + 1) * 64, oc * 512:(oc + 1) * 512],
                            )
                    else:
                        nc.sync.dma_start(
                            out_d[t * P:(t + 1) * P, oc * 512:(oc + 1) * 512],
                            ob[:, oc * 512:(oc + 1) * 512],
                        )

            # ---- filler closures ----
            def pj(kind, dst, w, m, tck, k0, k1):
                return lambda: emit_proj_piece(kind, dst, w, m, tck, k0, k1)

            def norm2(h0, h1, qc, act=False):
                def f():
                    emit_norm(h0, qc, act=act)
                    emit_norm(h1, qc, act=act)
                return f

            def norm2_pre(h0, h1, qc):
                def f():
                    emit_norm_pre(h0, qc)
                    emit_norm_pre(h1, qc)
                return f

            def norm2_post(h0, h1, qc):
                def f():
                    emit_norm_post(h0, qc)
                    emit_norm_post(h1, qc)
                return f

            def qch(m, c, half):
                return pj("q", qT, wqA, m, c, half * 4, half * 4 + 4)

            def kch(m, c, half):
                return pj("k", kTt, wkA, m, c, half * 4, half * 4 + 4)

            def vch(t):
                return lambda: emit_v_chain(t)

            def pvp(pair, j1):
                return lambda: emit_pv_steps(pair, j1)

            def opj(t, **kw):
                return lambda: emit_oproj(t, **kw)

            P01 = [(0, 0), (1, 0)]; P11 = [(0, 1), (1, 1)]
            P02 = [(0, 2), (1, 2)]; P03 = [(0, 3), (1, 3)]
            P20 = [(2, 0), (3, 0)]; P21 = [(2, 1), (3, 1)]
            P22 = [(2, 2), (3, 2)]; P23 = [(2, 3), (3, 3)]

            fill = [[] for _ in range(128)]

            def setf(u, *fns):
                fill[u].extend(fns)

            # Phase A (u0-15, m0/qc0): k(0,c1..3) paced ahead of the kt loop,
            # q(0,1) for phase B, then v0-7 (wv lands ~15us)
            A = [kch(0, 1, 0), kch(0, 1, 1), qch(0, 1, 0), qch(0, 1, 1),
                 kch(0, 2, 0), kch(0, 2, 1), kch(0, 3, 0), kch(0, 3, 1),
                 vch(0), vch(1), vch(2), vch(3), vch(4), vch(5), vch(6),
                 vch(7)]
            for i, f in enumerate(A):
                setf(i, f)
            def layout(base, pair, first, others, post=None):
                """slot0 = first (e.g. norm DVE-half of the PREVIOUS pair --
                must precede this pair's psum allocations on the same tag);
                slot2 = post (the norm's br+mul, by which time the DVE
                reciprocal chain has drained); odd slots = 8 PV pieces;
                even slots 4.. = others."""
                if first is not None:
                    setf(base, first)
                for i in range(8):
                    setf(base + 2 * i + 1, pvp(pair, 2 * i + 2))
                off = 1
                if post is not None:
                    setf(base + 2, post)
                    off = 2
                for j, f in enumerate(others):
                    setf(base + 2 * (j + off), f)

            # Phase B (u16-31, m0/qc1): v8-15 first, then PV pair h0/h1 qc0
            # in 4-kt strides AFTER the v-chains each stride reads
            setf(16, vch(8))
            setf(17, pvp(P01, 4))          # kt0-3: vext from phase A
            setf(18, vch(9))
            setf(19, vch(10))
            setf(20, vch(11))
            setf(21, pvp(P01, 8))          # kt4-7: vext4-7 from phase A
            setf(22, vch(12))
            setf(23, vch(13))
            setf(24, vch(14))
            setf(25, vch(15))
            setf(26, pvp(P01, 12))         # kt8-11: v8-11 at u16-20
            setf(27, qch(0, 2, 0))
            setf(28, qch(0, 2, 1))
            setf(29, pvp(P01, 16))         # kt12-15: v12-15 at u22-25

            def seq(*fns):
                return lambda: [f() for f in fns]

            # Phase C (u32-47): norms(qc0), PV qc1, q(0,3), k(1,0/1)
            layout(32, P11, norm2_pre(0, 1, 0),
                   [qch(0, 3, 0), qch(0, 3, 1), kch(1, 0, 0), kch(1, 0, 1),
                    kch(1, 1, 0), kch(1, 1, 1)], post=norm2_post(0, 1, 0))
            # Phase D (u48-63): PV qc2 + norms(qc1) + q(1,0), q(1,1), k(1,2)
            layout(48, P02, norm2_pre(0, 1, 1),
                   [qch(1, 0, 0), qch(1, 0, 1), qch(1, 1, 0), qch(1, 1, 1),
                    kch(1, 2, 0), kch(1, 2, 1)], post=norm2_post(0, 1, 1))
            # Phase E (u64-79, m1/qc0): PV qc3 + norms(qc2) + k(1,3), q(1,2/3)
            layout(64, P03, norm2_pre(0, 1, 2),
                   [kch(1, 3, 0), kch(1, 3, 1), qch(1, 2, 0), qch(1, 2, 1),
                    qch(1, 3, 0), qch(1, 3, 1)], post=norm2_post(0, 1, 2))
            # Phase F (u80-95, m1/qc1): PV h2/h3 qc0 + norms(h0/h1 qc3)
            layout(80, P20, norm2_pre(0, 1, 3), [],
                   post=norm2_post(0, 1, 3))
            # Phase G (u96-111, m1/qc2): PV h2/h3 qc1 + norms(h2h3 qc0) + O(0-3)
            layout(96, P21, norm2_pre(2, 3, 0),
                   [opj(0), opj(1), opj(2), opj(3)], post=norm2_post(2, 3, 0))
            # Phase H (u112-127): PV h2/h3 qc2 + norms(h2h3 qc1) + O(4-7)
            layout(112, P22, norm2_pre(2, 3, 1),
                   [opj(4), opj(5), opj(6), opj(7)], post=norm2_post(2, 3, 1))
            setf(127, norm2_pre(2, 3, 2))
            setf(127, pvp(P23, 2))

            # ---- emission ----
            # HAM warm-up: ~16 throwaway matmuls run while input DMA streams,
            # flipping the PE clock gate to 8/8 before the first real chain.
            wps = accp.tile([HD, 512], F32, tag="wk", name="wps")
            for i in range(10):
                nc.tensor.matmul(wps[:], lhsT=ones64[:], rhs=wrow[:],
                                 start=True, stop=True)

            emit_proj_piece("q", qT, wqA, 0, 0, 0, 4)
            emit_proj_piece("q", qT, wqA, 0, 0, 4, 8)
            emit_proj_piece("k", kTt, wkA, 0, 0, 0, 4)
            emit_proj_piece("k", kTt, wkA, 0, 0, 4, 8)

            units = [(m, kt, qc) for m in range(2) for qc in range(QC)
                     for kt in range(TBLK)]
            for u, (m, kt, qc) in enumerate(units):
                emit_S(m, kt, qc)
                for fn in fill[u]:
                    fn()

            # ---- tail: finish h2/h3 qc3 chains, last norms, O(8-15).
            # O blocks alternate psum tags and evict on ACT+DVE in parallel
            # (both engines idle post-exp) so the PE never waits on a slot.
            emit_norm_post(2, 2)
            emit_norm_post(3, 2)
            emit_oproj(8, evict_act="both", tag="wk")
            emit_oproj(9, evict_act="both", tag="wk")
            emit_pv_steps(P23, 8)
            emit_oproj(10, evict_act="both", tag="wk")
            emit_pv_steps(P23, 12)
            emit_oproj(11, evict_act="both", tag="wk")
            emit_pv_steps(P23, 16)
            emit_norm_pre(2, 3, act=True)
            emit_norm_pre(3, 3, act=True)
            emit_norm_post(2, 3)
            emit_norm_post(3, 3)
            emit_oproj(12, evict_act="both", tag="pv", split_dma=True)
            emit_oproj(13, evict_act="both", tag="wk", split_dma=True)
            emit_oproj(14, evict_act="both", tag="pv", split_dma=True)
            emit_oproj(15, evict_act="both", tag="wk", split_dma=True)
    nc.compile()
    return nc


_NC = None


def _get_nc():
    global _NC
    if _NC is None:
        _NC = _build()
    return _NC


def _pack(a, ktiles):
    """[ktiles*128, W] -> [128, ktiles*W]: tile k's rows land at cols k*W."""
    kt, w = ktiles, a.shape[1]
    return np.ascontiguousarray(
        a.reshape(kt, P, w).transpose(1, 0, 2).reshape(P, kt * w))


def _shard(inputs):
    x = np.asarray(inputs["x"], dtype=np.float32)
    W_q = np.asarray(inputs["W_q"], dtype=np.float32)
    W_k = np.asarray(inputs["W_k"], dtype=np.float32)
    W_v = np.asarray(inputs["W_v"], dtype=np.float32)
    W_o = np.asarray(inputs["W_o"], dtype=np.float32)
    bf = ml_dtypes.bfloat16
    in_maps = []
    for core in range(8):
        b, g = core // 4, core % 4
        sl = slice(g * GW, (g + 1) * GW)
        xTb = x[b].T.astype(bf)                      # [D, L]
        im = {
            "wqp": _pack(W_q[sl, :].T.astype(bf), KT),
            "wkp": _pack(W_k[sl, :].T.astype(bf), KT),
            "wvp": _pack(W_v[sl, :].T.astype(bf), KT),
            "wop": _pack(W_o[:, sl].T.astype(bf), 2),
        }
        for c in range(QC):
            im[f"xq{c}"] = _pack(xTb[:, c * 512:(c + 1) * 512], KT)
        in_maps.append(im)
    return in_maps


def _run(inputs, trace=False):
    nc = _get_nc()
    in_maps = _shard(inputs)
    res = run_bass_kernel_spmd(nc, in_maps, core_ids=list(range(8)), trace=trace)
    out = np.zeros((B, L, D), dtype=np.float32)
    for core in range(8):
        out[core // 4] += res.results[core]["out"].astype(np.float32)
    return out, res


def kernel(**inputs) -> np.ndarray:
    out, _ = _run(inputs, trace=False)
    return out


# revision 44
# speedup vs baseline: 1.0001x; 1.0001x over previous
"""Trainium2 Bass kernel for multi-head attention (B=2, L=2048, D=1024, H=16).

Sharding: 8 cores = 2 (batch) x 4 (head-groups of 4 heads).  Each core
computes q/k/v projections for its 4 heads, softmax attention, and a
partial output projection against its 256 columns of W_o.  The all-reduce
of the 4 partials per batch happens on the host (free).

v12: the exp stream on ACT (~133us busy) paces the kernel.  Each of 128
score units (pair m, key-tile kt, query-chunk qc) packs BOTH heads of the
pair into one [128,1024] PSUM tile -- even head on PE row-group 0 writing
cols 0-511, odd head on row-group 64 writing cols 512-1023 (K=64 matmuls
run concurrently on the two array halves; tile_position auto-derived from
base partitions) -- then one 1024-wide exp.  ~0.8us of PV / projection /
output chain work fills the PE between units.  x arrives as 4 contiguous
column-chunk tensors so the first score fires at ~6us.  PV keeps the
ones-column trick (out row 64 = softmax denominators).
"""

import sys

if "/opt/trn_rl_repo" not in sys.path:
    sys.path.insert(0, "/opt/trn_rl_repo")

import numpy as np
import ml_dtypes

import concourse.bass as bass
import concourse.mybir as mybir
import concourse.tile as tile
from concourse import bacc
from concourse.bass_utils import run_bass_kernel_spmd

B, L, D, H = 2, 2048, 1024, 16
HD = D // H          # 64 head dim
NH = 4               # heads per core
GW = NH * HD         # 256 group width
SCALE = (H / D) ** 0.5  # 1/8
P = 128
KT = D // P          # 8 contraction tiles over D
TBLK = L // P        # 16 token blocks of 128
QC = L // 512        # 4 query chunks of 512
BF16 = mybir.dt.bfloat16
F32 = mybir.dt.float32
EXP = mybir.ActivationFunctionType.Exp

PEXP_BUFS = 40       # [P,1024] pp tiles (1 per unit; consumed <=20 units later)


def _build():
    nc = bacc.Bacc(None, target_bir_lowering=False, debug=False)

    # Host-packed inputs: one row-contiguous DRAM tensor per SBUF tile so
    # each loads with a single dma_start of 4KB+ per-partition descriptors.
    xq_d = [nc.dram_tensor(f"xq{c}", (P, KT * 512), BF16, kind="ExternalInput")
            for c in range(QC)]
    wq_d = nc.dram_tensor("wqp", (P, KT * GW), BF16, kind="ExternalInput")
    wk_d = nc.dram_tensor("wkp", (P, KT * GW), BF16, kind="ExternalInput")
    wv_d = nc.dram_tensor("wvp", (P, KT * GW), BF16, kind="ExternalInput")
    wo_d = nc.dram_tensor("wop", (P, 2 * D), BF16, kind="ExternalInput")
    out_d = nc.dram_tensor("out", (L, D), BF16, kind="ExternalOutput")

    with tile.TileContext(nc) as tc:
        with (
            tc.tile_pool(name="persist", bufs=1) as pers,
            tc.tile_pool(name="pexp", bufs=PEXP_BUFS) as pexp,
            tc.tile_pool(name="oeT", bufs=4) as oep,
            tc.tile_pool(name="rcp", bufs=2) as rcpp,
            tc.tile_pool(name="srow", bufs=2) as srp,
            tc.tile_pool(name="osb", bufs=3) as osbp,
            tc.tile_pool(name="spsum", bufs=2, space="PSUM") as sps,
            tc.tile_pool(name="accp", bufs=2, space="PSUM") as accp,
        ):
            # ---- persistent SBUF tensors ----
            # xQ[c][:, k*512:(k+1)*512] = x^T rows k*128..+128, cols c*512..+512
            xQ = [pers.tile([P, KT * 512], BF16, tag=f"xQ{c}", name=f"xQ{c}")
                  for c in range(QC)]
            wqA = pers.tile([P, KT * GW], BF16, tag="wqA")
            wkA = pers.tile([P, KT * GW], BF16, tag="wkA")
            wvA = pers.tile([P, KT * GW], BF16, tag="wvA")
            woA = pers.tile([P, 2 * D], BF16, tag="woA")
            qT = [pers.tile([P, L], BF16, tag=f"qT{m}", name=f"qT{m}") for m in range(GW // P)]
            kTt = [pers.tile([P, L], BF16, tag=f"kT{m}", name=f"kT{m}") for m in range(GW // P)]
            vext = [pers.tile([P, NH * (HD + 1)], BF16, tag=f"vx{t}", name=f"vx{t}") for t in range(TBLK)]
            aoT = [pers.tile([P, L], BF16, tag=f"aoT{m}", name=f"aoT{m}") for m in range(GW // P)]
            ones64 = pers.tile([1, HD], BF16, tag="ones64")
            nc.any.memset(ones64[:], 1.0)
            warm = pers.tile([1, 2], BF16, tag="warm")
            nc.scalar.activation(warm[:], ones64[:, 0:2], EXP)  # preload exp table
            wrow = pers.tile([1, 512], BF16, tag="wrow")
            nc.vector.memset(wrow[:], 1.0)
            # ones columns of vext (col 64 per head): constant, disjoint from
            # the v-data the chains write -- set them all during the DMA wait
            for t in range(TBLK):
                vv = vext[t][:].rearrange("p (h e) -> p h e", h=NH)
                nc.vector.memset(vv[:, :, HD:HD + 1], 1.0)

            # ---- input DMA: 9 wide transfers; xq0 split in half so the
            # prefix q/k chains start on k-tiles 0-3 at half-arrival ----
            nc.sync.dma_start(wqA[:], wq_d[:])
            nc.sync.dma_start(wkA[:], wk_d[:])
            nc.sync.dma_start(xQ[0][:, 0:4 * 512], xq_d[0][:, 0:4 * 512])
            nc.sync.dma_start(xQ[0][:, 4 * 512:], xq_d[0][:, 4 * 512:])
            for c in range(1, QC):
                nc.sync.dma_start(xQ[c][:], xq_d[c][:])
            nc.sync.dma_start(wvA[:], wv_d[:])
            nc.sync.dma_start(woA[:], wo_d[:])

            # pp tiles: pp2[(m, kt, qc)] -> [128, 1024] (even head cols 0-511)
            pp2 = {}
            pv_ps = {}   # open PV chains: (h, qc) -> psum tile, +"next" kt
            pj_ps = {}   # open projection chains

            # ---- emitters ----
            def emit_S(m, kt, qc):
                """Both heads' scores for (kt, qc) into one PSUM tile, 1 exp."""
                ps = sps.tile([P, 1024], F32, tag="sc", name=f"sc{m}_{kt}_{qc}")
                for e in range(2):
                    off = e * HD
                    nc.tensor.matmul(
                        ps[:, e * 512:(e + 1) * 512],
                        lhsT=kTt[m][off:off + HD, kt * P:(kt + 1) * P],
                        rhs=qT[m][off:off + HD, qc * 512:(qc + 1) * 512],
                        start=True,
                        stop=True,
                    )
                pp = pexp.tile([P, 1024], BF16, tag="pp", name=f"pp{m}_{kt}_{qc}")
                nc.scalar.activation(pp[:], ps[:], EXP, scale=SCALE)
                pp2[(m, kt, qc)] = pp

            def emit_proj_piece(kind, dst, w, m, tck, k0, k1):
                """Piece [k0,k1) of an 8-matmul projection chain; evict at end."""
                key = (kind, m, tck)
                ps = pj_ps.get(key)
                if ps is None:
                    ps = accp.tile([P, 512], F32, tag="wk", name=f"pj_{kind}{m}_{tck}")
                    pj_ps[key] = ps
                for k in range(k0, k1):
                    nc.tensor.matmul(
                        ps[:],
                        lhsT=w[:, k * GW + m * P:k * GW + (m + 1) * P],
                        rhs=xQ[tck][:, k * 512:(k + 1) * 512],
                        start=(k == 0),
                        stop=(k == KT - 1),
                    )
                if k1 == KT:
                    nc.vector.tensor_copy(dst[m][:, tck * 512:(tck + 1) * 512], ps[:])
                    del pj_ps[key]

            def emit_v_chain(t):
                ps = accp.tile([P, 512], F32, tag="wk", name=f"vc{t}")
                c, tc = t // 4, t % 4
                for k in range(KT):
                    nc.tensor.matmul(
                        ps[:, :GW],
                        lhsT=xQ[c][:, k * 512 + tc * P:k * 512 + (tc + 1) * P],
                        rhs=wvA[:, k * GW:(k + 1) * GW],
                        start=(k == 0),
                        stop=(k == KT - 1),
                    )
                vv = vext[t][:].rearrange("p (h e) -> p h e", h=NH)
                pv = ps[:, :GW].rearrange("p (h e) -> p h e", h=NH)
                nc.vector.tensor_copy(vv[:, :, 0:HD], pv)

            def emit_pv_steps(targets, kt_hi):
                """Advance PV chains (h, qc) to key-tile kt_hi (exclusive)."""
                for h, qc in targets:
                    start_kt = pv_ps.get((h, qc, "next"), 0)
                    if start_kt >= kt_hi:
                        continue
                    ov = pv_ps.get((h, qc))
                    if ov is None:
                        ov = accp.tile([HD + 1, 512], F32, tag="pv",
                                       name=f"ov{h}_{qc}")
                        pv_ps[(h, qc)] = ov
                    for k in range(start_kt, kt_hi):
                        nc.tensor.matmul(
                            ov[:],
                            lhsT=vext[k][:, h * (HD + 1):(h + 1) * (HD + 1)],
                            rhs=pp2[(h // 2, k, qc)][:, (h % 2) * 512:(h % 2 + 1) * 512],
                            start=(k == 0),
                            stop=(k == TBLK - 1),
                        )
                    pv_ps[(h, qc, "next")] = kt_hi

            norm_st = {}

            def emit_norm_pre(h, qc, act=False):
                """DVE half of the normalization: evict ov, build 1/sums row.
                Split from the br matmul so the PE queue never head-blocks on
                the DVE reciprocal chain."""
                ov = pv_ps.pop((h, qc))
                pv_ps.pop((h, qc, "next"), None)
                oe = oep.tile([HD, 512], BF16, tag="oe")
                if act:
                    nc.scalar.copy(oe[:], ov[0:HD, :])
                else:
                    nc.vector.tensor_copy(oe[:], ov[0:HD, :])
                srow = srp.tile([1, 512], F32, tag="s")
                nc.vector.tensor_copy(srow[:], ov[HD:HD + 1, :])
                rr = rcpp.tile([1, 512], F32, tag="r")
                nc.vector.reciprocal_approx_fast(rr[:], srow[:])
                rrb = rcpp.tile([1, 512], BF16, tag="rb")
                nc.vector.tensor_copy(rrb[:], rr[:])
                norm_st[(h, qc)] = (oe, rrb)

            def emit_norm_post(h, qc):
                """PE broadcast of 1/sums + the final DVE multiply into aoT."""
                oe, rrb = norm_st.pop((h, qc))
                m, off = h // 2, (h % 2) * HD
                br = accp.tile([HD, 512], F32, tag="wk", name=f"br{h}_{qc}")
                nc.tensor.matmul(br[:], lhsT=ones64[:], rhs=rrb[:], start=True, stop=True)
                nc.vector.tensor_mul(
                    aoT[m][off:off + HD, qc * 512:(qc + 1) * 512],
                    oe[:],
                    br[:],
                )

            def emit_norm(h, qc, act=False):
                emit_norm_pre(h, qc, act=act)
                emit_norm_post(h, qc)

            def emit_oproj(t, evict_act=False, split_dma=False, tag="wk"):
                ob = osbp.tile([P, D], BF16, tag="ob")
                for oc in range(2):
                    ps = accp.tile([P, 512], F32, tag=tag, name=f"op{t}_{oc}")
                    for i in range(GW // P):
                        nc.tensor.matmul(
                            ps[:],
                            lhsT=aoT[i][:, t * P:(t + 1) * P],
                            rhs=woA[:, i * D + oc * 512:i * D + (oc + 1) * 512],
                            start=(i == 0),
                            stop=(i == GW // P - 1),
                        )
                    if evict_act == "both":
                        if oc == 0:
                            nc.scalar.copy(ob[:, 0:512], ps[:])
                        else:
                            nc.vector.tensor_copy(ob[:, 512:1024], ps[:])
                    elif evict_act:
                        nc.scalar.copy(ob[:, oc * 512:(oc + 1) * 512], ps[:])
                    else:
                        nc.vector.tensor_copy(ob[:, oc * 512:(oc + 1) * 512], ps[:])
                    if split_dma:
                        for g in range(2):
                            nc.sync.dma_start(
                                out_d[t * P + g * 64:t * P + (g + 1) * 64,
                                      oc * 512:(oc + 1) * 512],
                                ob[g * 64:(g + 1) * 64, oc * 512:(oc + 1) * 512],
                            )
                    else:
                        nc.sync.dma_start(
                            out_d[t * P:(t + 1) * P, oc * 512:(oc + 1) * 512],
                            ob[:, oc * 512:(oc + 1) * 512],
                        )

            # ---- filler closures ----
            def pj(kind, dst, w, m, tck, k0, k1):
                return lambda: emit_proj_piece(kind, dst, w, m, tck, k0, k1)

            def norm2(h0, h1, qc, act=False):
                def f():
                    emit_norm(h0, qc, act=act)
                    emit_norm(h1, qc, act=act)
                return f

            def norm2_pre(h0, h1, qc):
                def f():
                    emit_norm_pre(h0, qc)
                    emit_norm_pre(h1, qc)
                return f

            def norm2_post(h0, h1, qc):
                def f():
                    emit_norm_post(h0, qc)
                    emit_norm_post(h1, qc)
                return f

            def qch(m, c, half):
                return pj("q", qT, wqA, m, c, half * 4, half * 4 + 4)

            def kch(m, c, half):
                return pj("k", kTt, wkA, m, c, half * 4, half * 4 + 4)

            def vch(t):
                return lambda: emit_v_chain(t)

            def pvp(pair, j1):
                return lambda: emit_pv_steps(pair, j1)

            def opj(t, **kw):
                return lambda: emit_oproj(t, **kw)

            P01 = [(0, 0), (1, 0)]; P11 = [(0, 1), (1, 1)]
            P02 = [(0, 2), (1, 2)]; P03 = [(0, 3), (1, 3)]
            P20 = [(2, 0), (3, 0)]; P21 = [(2, 1), (3, 1)]
            P22 = [(2, 2), (3, 2)]; P23 = [(2, 3), (3, 3)]

            fill = [[] for _ in range(128)]

            def setf(u, *fns):
                fill[u].extend(fns)

            # Phase A (u0-15, m0/qc0): k(0,c1..3) paced ahead of the kt loop,
            # q(0,1) for phase B, then v0-7 (wv lands ~15us)
            A = [kch(0, 1, 0), kch(0, 1, 1), qch(0, 1, 0), qch(0, 1, 1),
                 kch(0, 2, 0), kch(0, 2, 1), kch(0, 3, 0), kch(0, 3, 1),
                 vch(0), vch(1), vch(2), vch(3), vch(4), vch(5), vch(6),
                 vch(7)]
            for i, f in enumerate(A):
                setf(i, f)
            def layout(base, pair, first, others, post=None):
                """slot0 = first (e.g. norm DVE-half of the PREVIOUS pair --
                must precede this pair's psum allocations on the same tag);
                slot2 = post (the norm's br+mul, by which time the DVE
                reciprocal chain has drained); odd slots = 8 PV pieces;
                even slots 4.. = others."""
                if first is not None:
                    setf(base, first)
                for i in range(8):
                    setf(base + 2 * i + 1, pvp(pair, 2 * i + 2))
                off = 1
                if post is not None:
                    setf(base + 2, post)
                    off = 2
                for j, f in enumerate(others):
                    setf(base + 2 * (j + off), f)

            # Phase B (u16-31, m0/qc1): v8-15 first, then PV pair h0/h1 qc0
            # in 4-kt strides AFTER the v-chains each stride reads
            setf(16, vch(8))
            setf(17, pvp(P01, 4))          # kt0-3: vext from phase A
            setf(18, vch(9))
            setf(19, vch(10))
            setf(20, vch(11))
            setf(21, pvp(P01, 8))          # kt4-7: vext4-7 from phase A
            setf(22, vch(12))
            setf(23, vch(13))
            setf(24, vch(14))
            setf(25, vch(15))
            setf(26, pvp(P01, 12))         # kt8-11: v8-11 at u16-20
            setf(27, qch(0, 2, 0))
            setf(28, qch(0, 2, 1))
            setf(29, pvp(P01, 16))         # kt12-15: v12-15 at u22-25

            def seq(*fns):
                return lambda: [f() for f in fns]

            # Phase C (u32-47): norms(qc0), PV qc1, q(0,3), k(1,0/1)
            layout(32, P11, norm2_pre(0, 1, 0),
                   [qch(0, 3, 0), qch(0, 3, 1), kch(1, 0, 0), kch(1, 0, 1),
                    kch(1, 1, 0), kch(1, 1, 1)], post=norm2_post(0, 1, 0))
            # Phase D (u48-63): PV qc2 + norms(qc1) + q(1,0), q(1,1), k(1,2)
            layout(48, P02, norm2_pre(0, 1, 1),
                   [qch(1, 0, 0), qch(1, 0, 1), qch(1, 1, 0), qch(1, 1, 1),
                    kch(1, 2, 0), kch(1, 2, 1)], post=norm2_post(0, 1, 1))
            # Phase E (u64-79, m1/qc0): PV qc3 + norms(qc2) + k(1,3), q(1,2/3)
            layout(64, P03, norm2_pre(0, 1, 2),
                   [kch(1, 3, 0), kch(1, 3, 1), qch(1, 2, 0), qch(1, 2, 1),
                    qch(1, 3, 0), qch(1, 3, 1)], post=norm2_post(0, 1, 2))
            # Phase F (u80-95, m1/qc1): PV h2/h3 qc0 + norms(h0/h1 qc3)
            layout(80, P20, norm2_pre(0, 1, 3), [],
                   post=norm2_post(0, 1, 3))
            # Phase G (u96-111, m1/qc2): PV h2/h3 qc1 + norms(h2h3 qc0) + O(0-3)
            layout(96, P21, norm2_pre(2, 3, 0),
                   [opj(0), opj(1), opj(2), opj(3)], post=norm2_post(2, 3, 0))
            # Phase H (u112-127): PV h2/h3 qc2 + norms(h2h3 qc1) + O(4-7)
            layout(112, P22, norm2_pre(2, 3, 1),
                   [opj(4), opj(5), opj(6), opj(7)], post=norm2_post(2, 3, 1))
            setf(127, norm2_pre(2, 3, 2))
            setf(127, pvp(P23, 2))

            # ---- emission ----
            # HAM warm-up: ~16 throwaway matmuls run while input DMA streams,
            # flipping the PE clock gate to 8/8 before the first real chain.
            wps = accp.tile([HD, 512], F32, tag="wk", name="wps")
            for i in range(12):
                nc.tensor.matmul(wps[:], lhsT=ones64[:], rhs=wrow[:],
                                 start=True, stop=True)

            emit_proj_piece("q", qT, wqA, 0, 0, 0, 4)
            emit_proj_piece("q", qT, wqA, 0, 0, 4, 8)
            emit_proj_piece("k", kTt, wkA, 0, 0, 0, 4)
            emit_proj_piece("k", kTt, wkA, 0, 0, 4, 8)

            units = [(m, kt, qc) for m in range(2) for qc in range(QC)
                     for kt in range(TBLK)]
            for u, (m, kt, qc) in enumerate(units):
                emit_S(m, kt, qc)
                for fn in fill[u]:
                    fn()

            # ---- tail: finish h2/h3 qc3 chains, last norms, O(8-15).
            # O blocks alternate psum tags and evict on ACT+DVE in parallel
            # (both engines idle post-exp) so the PE never waits on a slot.
            emit_norm_post(2, 2)
            emit_norm_post(3, 2)
            emit_oproj(8, evict_act="both", tag="wk", split_dma=True)
            emit_oproj(9, evict_act="both", tag="wk", split_dma=True)
            emit_pv_steps(P23, 8)
            emit_oproj(10, evict_act="both", tag="wk", split_dma=True)
            emit_pv_steps(P23, 12)
            emit_oproj(11, evict_act="both", tag="wk", split_dma=True)
            emit_pv_steps(P23, 16)
            emit_norm_pre(2, 3, act=True)
            emit_norm_pre(3, 3, act=True)
            emit_norm_post(2, 3)
            emit_norm_post(3, 3)
            emit_oproj(12, evict_act="both", tag="pv", split_dma=True)
            emit_oproj(13, evict_act="both", tag="wk", split_dma=True)
            emit_oproj(14, evict_act="both", tag="pv", split_dma=True)
            emit_oproj(15, evict_act="both", tag="wk", split_dma=True)
    nc.compile()
    return nc


_NC = None


def _get_nc():
    global _NC
    if _NC is None:
        _NC = _build()
    return _NC


def _pack(a, ktiles):
    """[ktiles*128, W] -> [128, ktiles*W]: tile k's rows land at cols k*W."""
    kt, w = ktiles, a.shape[1]
    return np.ascontiguousarray(
        a.reshape(kt, P, w).transpose(1, 0, 2).reshape(P, kt * w))


def _shard(inputs):
    x = np.asarray(inputs["x"], dtype=np.float32)
    W_q = np.asarray(inputs["W_q"], dtype=np.float32)
    W_k = np.asarray(inputs["W_k"], dtype=np.float32)
    W_v = np.asarray(inputs["W_v"], dtype=np.float32)
    W_o = np.asarray(inputs["W_o"], dtype=np.float32)
    bf = ml_dtypes.bfloat16
    in_maps = []
    for core in range(8):
        b, g = core // 4, core % 4
        sl = slice(g * GW, (g + 1) * GW)
        xTb = x[b].T.astype(bf)                      # [D, L]
        im = {
            "wqp": _pack(W_q[sl, :].T.astype(bf), KT),
            "wkp": _pack(W_k[sl, :].T.astype(bf), KT),
            "wvp": _pack(W_v[sl, :].T.astype(bf), KT),
            "wop": _pack(W_o[:, sl].T.astype(bf), 2),
        }
        for c in range(QC):
            im[f"xq{c}"] = _pack(xTb[:, c * 512:(c + 1) * 512], KT)
        in_maps.append(im)
    return in_maps


def _run(inputs, trace=False):
    nc = _get_nc()
    in_maps = _shard(inputs)
    res = run_bass_kernel_spmd(nc, in_maps, core_ids=list(range(8)), trace=trace)
    out = np.zeros((B, L, D), dtype=np.float32)
    for core in range(8):
        out[core // 4] += res.results[core]["out"].astype(np.float32)
    return out, res


def kernel(**inputs) -> np.ndarray:
    out, _ = _run(inputs, trace=False)
    return out
